# revision 65
# baseline (speedup 1.0000x reference)
"""Multi-head attention kernel for Trainium2, sharded over 8 NeuronCores.

Sharding: data parallel over batch (B=2 -> 4 cores each) x tensor parallel
over heads (12 heads -> 3 heads per core). Per-head partial output
projections are summed on the host (the all-reduce of the tensor-parallel
hint) and the output bias added there.

Engine plan (v3 — fp16 everywhere, dual-engine exp, flipped probs@V):
  - all matmul operands are fp16 (1 cyc/col on the PE, same rate as fp32r,
    half the SBUF/DMA bytes). fp8 variants of any tensor feeding the
    softmax-weighted sum were measured 2-3x OVER the 2e-2 gate: quantization
    noise on q/k/v/probs does NOT average away under softmax, because the
    weighted-mean signal and the weighted-mean noise shrink identically.
  - the key bias is dropped entirely: softmax over keys is invariant to the
    per-query constant q.bk, so only the query bias bq survives (it rides
    the forced PSUM->SBUF conversion as a per-partition scalar add).
  - exp splits across BOTH the Activation engine (native Exp -> fp16) and
    the Vector engine (Schraudolph int16 trick: bits = rint(184.665*s_raw
    + 15315) bitcast fp16 ~ exp(s_raw/8), one tensor_scalar op, ~1.7% rms),
    interleaved within each block so both engines run concurrently.
  - probs@V runs FLIPPED: [128,128] probs blocks are the stationary operand
    (LdWeights is free) and the [64 v | ones] fp16 tile streams 65 moving
    columns -> 50k PE cycles instead of 98k for the probs-moving
    orientation. The ones column lands the softmax denominator at psum
    column 64 = per-QUERY-partition, so normalization is one reciprocal +
    one broadcast-multiply per block.
  - the normalized ctx [query, (head,dh)] is transposed back to [dh, query]
    for the output projection via PE permutation matmuls (is_transpose with
    an identity moving operand, 128 cycles per [128,128] block) + engine
    copies. (An SBUF->SBUF dma_start_transpose variant measured faster on
    paper but was flaky on the execution path; PE transposes are stable.)
  - out tiles are copied PSUM->SBUF as fp16 by the Activation engine and
    DMA'd at half the bytes; the host gather sums the 4 tensor-parallel
    partials in fp32 and adds the output bias.
"""

from collections import deque

import numpy as np
import ml_dtypes

import concourse.mybir as mybir
from concourse import bacc
from concourse.tile import TileContext
from concourse.bass_utils import run_bass_kernel_spmd

H, D, DH = 12, 768, 64
B, S = 2, 2048
NCORES = 8
CORES_PER_BATCH = 4
HPC = 3  # heads per core
SQ = 512  # query-chunk width
NSQ = S // SQ  # 4
NSK = S // 128  # 16 key chunks
NPD = 3  # d-chunk pairs (contraction 768 = 3 * 2 * 128)
NPK = NSK // 2  # 8 key-chunk pairs for probs@V DoubleRow

F32 = mybir.dt.float32
F32R = mybir.dt.float32r
F16 = mybir.dt.float16
F8E4 = mybir.dt.float8e4
F8E5 = mybir.dt.float8e5
I16 = mybir.dt.int16
ADD = mybir.AluOpType.add
MULT = mybir.AluOpType.mult
EXP = mybir.ActivationFunctionType.Exp
IDENT = mybir.ActivationFunctionType.Identity
DR = mybir.MatmulPerfMode.DoubleRow

# Schraudolph fp16 exp constants (round-to-nearest float->int16 verified):
# bits = rint(1024*log2(e)/8 * s_raw + bias) => fp16(bits) ~ exp(s_raw/8),
# max rel err 3.0%, ~1.7% rms -> ~0.6% on the final output (fp8 variants
# measured 2-3x over the 2e-2 absmax gate; quantization noise on probs/v
# does NOT average away under softmax - noise and signal shrink identically)
SCH_A = 184.6649652337873
SCH_B = 15315.0

# per-block exp engine pattern (A=Activation native exp, D=DVE int8 trick),
# interleaved so adjacent score tiles land on different engines
EXP_PATS = ["ADADAADA", "ADADADAD", "ADADADAD"]


NDC = D // 128  # 6


def _build_module():
    nc = bacc.Bacc("TRN2", target_bir_lowering=False, debug=False, num_devices=NCORES)
    xT16 = nc.declare_dram_parameter("xT16", [128, NDC, S], F16, isOutput=False)
    wqk16 = nc.declare_dram_parameter("wqk16", [128, HPC, NDC, 128], F16, isOutput=False)
    wv16 = nc.declare_dram_parameter("wv16", [128, NDC, HPC * DH], F16, isOutput=False)
    wo01 = nc.declare_dram_parameter("wo01", [128, D], F16, isOutput=False)
    wo2 = nc.declare_dram_parameter("wo2", [64, D], F16, isOutput=False)
    bq = nc.declare_dram_parameter("bq", [64, HPC], F32, isOutput=False)
    bv = nc.declare_dram_parameter("bv", [128, HPC * DH], F32, isOutput=False)
    ident = nc.declare_dram_parameter("ident", [128, 128], F16, isOutput=False)
    out = nc.declare_dram_parameter("out", [S, D], F16, isOutput=True)

    with TileContext(nc) as tc:
        _body(nc, tc, xT16, wqk16, wv16, wo01, wo2, bq, bv, ident, out)
    nc.compile()
    return nc


def _body(nc, tc, xT16, wqk16, wv16, wo01, wo2, bq, bv, ident, out):
    with (
        tc.tile_pool(name="persist", bufs=1) as P1,
        tc.tile_pool(name="work", bufs=4) as W2,
        tc.tile_pool(name="probs", bufs=2) as PR,
        # PSUM: ACC 2 banks rotating [128,512] tiles (the probs@V
        # accumulator cxp shares this rotation - it is bank-sized and lives
        # only through the in-block tail), SPS 3x[128,1024]: the 3rd score
        # buffer lets the PE pre-issue scores(j+2) so each exp engine runs
        # back-to-back instead of idling exp(j)+sem+scores per pair.
        tc.tile_pool(name="acc", bufs=2, space="PSUM") as ACC,
        tc.tile_pool(name="sps", bufs=3, space="PSUM") as SPS,
    ):
        xT16_sb = P1.tile([128, NDC, S], F16, tag="xT16")
        wqk_sb = P1.tile([128, HPC, NDC, 128], F16, tag="wqk")
        wv_sb = P1.tile([128, NDC, HPC * DH], F16, tag="wv")
        wo01_sb = P1.tile([128, D], F16, tag="wo01")
        wo2_sb = P1.tile([64, D], F16, tag="wo2")
        bq_sb = P1.tile([64, HPC], F32, tag="bq")
        bv_sb = P1.tile([128, HPC * DH], F32, tag="bv")
        id_sb = P1.tile([128, 128], F16, tag="ident")
        # q/k transposed per head, fp16 (scores run fp16 at 1 cyc/col)
        qT = [P1.tile([64, S], F16, tag=f"qT{h}", name=f"qT{h}") for h in range(HPC)]
        kT = [P1.tile([64, S], F16, tag=f"kT{h}", name=f"kT{h}") for h in range(HPC)]
        # v tiles fp16: per key chunk, per head [v_h (64) | ones-column] —
        # the ones column rides the probs@V moving operand and lands the
        # softmax denominator at psum column 64, i.e. per-QUERY-partition
        vp = P1.tile([128, NSK, HPC, 65], F16, tag="vp")

        # DMAs in first-needed order (dma_start costs ~0.6-1us serialized DGE
        # overhead each, so batch big except the first matmul's deps).
        nc.sync.dma_start(xT16_sb[:, :, 0:SQ], xT16[:, :, 0:SQ])
        nc.sync.dma_start(wqk_sb[:, 0, :, :], wqk16[:, 0, :, :])
        nc.sync.dma_start(bq_sb[:], bq[:])
        for sc in range(1, NSQ):
            nc.sync.dma_start(
                xT16_sb[:, :, sc * SQ:(sc + 1) * SQ],
                xT16[:, :, sc * SQ:(sc + 1) * SQ],
            )
        nc.sync.dma_start(wv_sb[:], wv16[:])
        nc.sync.dma_start(wqk_sb[:, 1:3, :, :], wqk16[:, 1:3, :, :])
        nc.sync.dma_start(bv_sb[:], bv[:])
        nc.sync.dma_start(wo01_sb[:], wo01[:])
        nc.sync.dma_start(wo2_sb[:], wo2[:])
        nc.sync.dma_start(id_sb[:], ident[:])
        # ones column next to each head's v block (softmax denominator trick)
        nc.gpsimd.memset(vp[:, :, :, 64:65], 1.0)

        def qk_unit(h, sc):
            # one query-chunk of q/k projection for head h (fp16 matmuls —
            # fp8 here puts ~5% correlated error on q that softmax cannot
            # average away), then PSUM->SBUF fp16 conversions.
            ps = ACC.tile([128, SQ], F32, tag="acc", name=f"qkps{h}_{sc}")
            for o in range(NDC):
                nc.tensor.matmul(
                    ps[:],
                    wqk_sb[:, h, o, :],
                    xT16_sb[:, o, sc * SQ:(sc + 1) * SQ],
                    start=(o == 0),
                    stop=(o == NDC - 1),
                )
            # q rows 0..63 with per-dim bias (DVE); k rows 64..127 plain (ACT)
            nc.vector.tensor_scalar(
                qT[h][:, sc * SQ:(sc + 1) * SQ], ps[0:64, :],
                bq_sb[0:64, h:h + 1], None, ADD,
            )
            nc.scalar.copy(kT[h][:, sc * SQ:(sc + 1) * SQ], ps[64:128, :])

        def v_unit(kc):
            # one key-chunk of v for all 3 heads (fp16 matmuls), then
            # bias-add conversion into the fp16 vp tile (DVE).
            ps = ACC.tile([128, SQ], F32, tag="acc", name=f"vps{kc}")
            psv = ps[:, 0:HPC * DH]
            for o in range(NDC):
                nc.tensor.matmul(
                    psv,
                    xT16_sb[:, o, kc * 128:(kc + 1) * 128],
                    wv_sb[:, o, :],
                    start=(o == 0),
                    stop=(o == NDC - 1),
                )
            nc.vector.tensor_tensor(
                vp[:, kc, :, 0:64],
                psv.rearrange("p (h m) -> p h m", m=DH),
                bv_sb[:].rearrange("p (h m) -> p h m", m=DH),
                ADD,
            )

        def proj_span(sc, ms, n0, nw, ot, ctx01, ctx2, last):
            # one column-span of the out-projection for row-tile ms: both
            # head-accumulation stages into ONE psum tile (peak 1 ACC slot)
            ops_t = ACC.tile([128, nw], F32, tag="acc", name=f"ops{sc}_{ms}_{n0}")
            nc.tensor.matmul(
                ops_t[:], ctx01[:, ms * 128:(ms + 1) * 128],
                wo01_sb[:, n0:n0 + nw], start=True, stop=False,
            )
            nc.tensor.matmul(
                ops_t[:], ctx2[0:64, ms * 128:(ms + 1) * 128],
                wo2_sb[:, n0:n0 + nw], start=False, stop=True,
            )
            nc.scalar.copy(ot[:, n0:n0 + nw], ops_t[:])
            if last:
                nc.sync.dma_start(
                    out[(sc * 4 + ms) * 128:(sc * 4 + ms + 1) * 128, :], ot[:]
                )

        filler = deque()
        # probs@V / normalize / transpose units of block b, woven into block
        # b+1 as PE/vector fillers: a PSUM start_tensor_calc zeroes a whole
        # 2KB bank, so the 4 query-chunk accumulation groups sharing the cxp
        # bank must run one-after-another (qc-outer), which needs the full
        # block's probs — hence the one-block lag.
        pending = deque()
        exp_cnt = [0]

        def attention_block(sc, h, ctxA, ctxB, pops_per_j=1, pop_stride=1,
                            pre_sps=None, nxt=None):
            # probs fp16 flat [128, NSK*SQ], key chunks on partitions; exp
            # runs 1024 wide per key-chunk pair, alternating ACT (native
            # exp) / DVE (int16 Schraudolph). probs@V runs FLIPPED: probs
            # [128,128] blocks are the stationary operand and the [64 v |
            # ones] tile streams 65 columns — half the PE cycles of the
            # probs-moving orientation (and the denominator lands per-QUERY-
            # partition at psum column 64, so normalization is one
            # broadcast-multiply instead of a partition-aligned divide).
            probs = PR.tile([128, NSK * SQ], F16, tag="probs", name=f"pr{sc}_{h}")
            # full-bank cxp so start_tensor_calc's bank-wide zero marking
            # stays within this tile
            cxp = ACC.tile([128, 4, 128], F32, tag="acc", name=f"cxp{sc}_{h}")

            def pvq(qc):
                # one query-window's full key accumulation (qc-outer: groups
                # sharing the bank must not interleave their start/stop)
                for mk in range(NSK):
                    nc.tensor.matmul(
                        cxp[:, qc, 0:65],
                        probs[:, mk * SQ + qc * 128:mk * SQ + (qc + 1) * 128],
                        vp[:, mk, h, :],
                        start=(mk == 0),
                        stop=(mk == NSK - 1),
                    )

            def normalize():
                # cols 0..63: raw ctx [query, dh]; col 64: denominators
                rinv = W2.tile([128, 4, 1], F32, tag="recip", name=f"r{sc}_{h}")
                nc.vector.reciprocal(rinv[:], cxp[:, :, 64:65])
                dst = (ctxA[:, :, h * 64:(h + 1) * 64] if h < 2
                       else ctxB[:, :, 0:64])
                nc.vector.scalar_tensor_tensor(
                    dst, cxp[:, :, 0:64], 1.0,
                    rinv[:].to_broadcast([128, 4, DH]), MULT, MULT,
                )

            npre = len(pre_sps) if pre_sps else 0
            for j in range(NPK):
                if j < npre:
                    sps = pre_sps[j]
                else:
                    sps = SPS.tile([128, 2 * SQ], F32, tag="sps",
                                   name=f"sps{sc}_{h}_{j}")
                    for half in range(2):
                        mk = 2 * j + half
                        nc.tensor.matmul(
                            sps[:, half * SQ:(half + 1) * SQ],
                            kT[h][:, mk * 128:(mk + 1) * 128],
                            qT[h][:, sc * SQ:(sc + 1) * SQ],
                            start=True,
                            stop=True,
                        )
                dst = probs[:, j * 2 * SQ:(j + 1) * 2 * SQ]
                # ACT/DVE interleaved within the block so both engines chew
                # concurrently through the 2-deep score-buffer pipeline;
                # 5:3 / 5:3 / 4:4 block cycle balances total load (~56A/40D)
                pat = EXP_PATS[(exp_cnt[0] // NPK) % len(EXP_PATS)]
                if pat[j] == "A":
                    nc.scalar.activation(dst, sps[:], EXP, scale=0.125)
                else:
                    nc.vector.tensor_scalar(
                        dst.bitcast(I16), sps[:], SCH_A, SCH_B, MULT, ADD
                    )
                exp_cnt[0] += 1
                if j % pop_stride == 0:
                    for _ in range(pops_per_j):
                        if filler:
                            filler.popleft()()
            hoisted = []
            for qc in range(4):
                pvq(qc)
                if qc in (1, 2, 3) and nxt is not None:
                    # hoist the NEXT block's first three score pairs (PE-only
                    # — their exps stay in that block's body): fills this
                    # block's PE tail and lets both exp engines launch at
                    # the next block's start. Touches only the 3-deep sps
                    # rotation and the long-lived qT/kT tiles (the
                    # scores-after-exp WAR the in-block loop already
                    # exercises); hoisting the EXP side corrupts.
                    nsc, nh = nxt
                    p = len(hoisted)
                    hs = SPS.tile([128, 2 * SQ], F32, tag="sps",
                                  name=f"hsps{nsc}_{nh}_{p}")
                    for half in range(2):
                        mk = 2 * p + half
                        nc.tensor.matmul(
                            hs[:, half * SQ:(half + 1) * SQ],
                            kT[nh][:, mk * 128:(mk + 1) * 128],
                            qT[nh][:, nsc * SQ:(nsc + 1) * SQ],
                            start=True,
                            stop=True,
                        )
                    hoisted.append(hs)
            normalize()
            return hoisted

        # PE warmup (p-state ramp) + ACT exp-table preload during DMA wait
        warm = P1.tile([64, 512], F32R, tag="warm")
        nc.vector.memset(warm[:].bitcast(F32), 0.0)
        wps = ACC.tile([128, 512], F32, tag="acc", name="warmps")
        for _ in range(10):
            nc.tensor.matmul(wps[:], warm[:, 0:128], warm[:], start=True, stop=True)
        wact = P1.tile([64, 1], F16, tag="wact")
        nc.scalar.activation(wact[:], warm[:, 0:1].bitcast(F32), EXP, scale=0.125)

        qk_unit(0, 0)

        ctxs = {}
        pre_state = {"sps": None}
        for sc in range(NSQ):
            ctxA = W2.tile([128, 4, 128], F16, tag="ctxA", name=f"cA_{sc}")
            ctxB = W2.tile([128, 4, 128], F16, tag="ctxB", name=f"cB_{sc}")
            c01T = W2.tile([128, SQ], F16, tag="c01T", name=f"c01T_{sc}")
            c2T = W2.tile([128, SQ], F16, tag="c2T", name=f"c2T_{sc}")
            ctxs[sc] = (c01T, c2T)

            def transpose(src, dstT, rows=128):
                # [128 q, qc, hd] -> [hd, qc-window of q] via PE permutation
                # matmuls (is_transpose) + PSUM->SBUF copies
                for qc in range(4):
                    tp = SPS.tile([rows, 128], F16, tag="sps",
                                  name=f"tp{sc}_{qc}_{rows}")
                    nc.tensor.matmul(tp[:], src[:, qc, 0:rows], id_sb[:],
                                     is_transpose=True)
                    if rows == 128:
                        nc.scalar.copy(
                            dstT[0:rows, qc * 128:(qc + 1) * 128], tp[:])
                    else:
                        nc.vector.tensor_copy(
                            dstT[0:rows, qc * 128:(qc + 1) * 128], tp[:])

            for h in range(HPC):
                pops = 1
                if sc == 0 and h == 0:
                    filler.append(lambda: qk_unit(0, 1))
                    filler.append(lambda: qk_unit(0, 2))
                    filler.append(lambda: qk_unit(0, 3))
                    for i in range(NSK // 2):
                        filler.append(lambda i=i: v_unit(2 * i))
                        filler.append(lambda i=i: v_unit(2 * i + 1))
                        if i % 2 == 0:
                            filler.append(lambda i=i: qk_unit(1, i // 2))
                    pops = 3
                elif sc == 0 and h == 1:
                    for i in range(NSQ):
                        filler.append(lambda i=i: qk_unit(2, i))
                stride = 3 if sc > 0 else (3 if h == 1 else 1)
                bidx = sc * HPC + h
                nxt = ((sc + (h + 1) // HPC, (h + 1) % HPC)
                       if bidx + 1 < NSQ * HPC else None)
                pre_state["sps"] = attention_block(
                    sc, h, ctxA, ctxB, pops_per_j=pops, pop_stride=stride,
                    pre_sps=pre_state["sps"], nxt=nxt)
                if h == 1:
                    transpose(ctxA, c01T)
                elif h == 2:
                    transpose(ctxB, c2T, rows=64)
            for ms in range(SQ // 128):
                ot = W2.tile([128, D], F16, tag="out", name=f"ot{sc}_{ms}")
                for n0, nw, last in ((0, 512, False), (512, 256, True)):
                    filler.append(
                        lambda sc=sc, ms=ms, n0=n0, nw=nw, ot=ot, last=last:
                        proj_span(sc, ms, n0, nw, ot, *ctxs[sc], last)
                    )
        while filler:
            filler.popleft()()


_CACHE = {}


def _get_module():
    if "nc" not in _CACHE:
        _CACHE["nc"] = _build_module()
    return _CACHE["nc"]


def make_in_maps(x, Wq, Wk, Wv, bq, bk, bv, Wo):
    f32 = np.float32
    f16 = np.float16
    f8 = ml_dtypes.float8_e4m3
    in_maps = []
    for c in range(NCORES):
        b = c // CORES_PER_BATCH
        hh = [HPC * (c % CORES_PER_BATCH) + i for i in range(HPC)]
        # xT16 [128, chunk, S]: d = chunk*128 + partition
        xt16 = x[b].T.reshape(NDC, 128, S).transpose(1, 0, 2)
        # wqk16 [128, head, chunk, 128]
        wqk = np.stack(
            [np.concatenate([Wq[h], Wk[h]], axis=1) for h in hh]
        )  # [3, 768, 128]
        wqk = wqk.reshape(HPC, NDC, 128, 128).transpose(2, 0, 1, 3)
        # wv16 [128, chunk, 192]
        wv_stack = np.concatenate([Wv[h] for h in hh], axis=1)  # [768, 192]
        wv_stack = wv_stack.reshape(NDC, 128, HPC * DH).transpose(1, 0, 2)
        in_maps.append({
            "xT16": np.ascontiguousarray(xt16).astype(f16),
            "wqk16": np.ascontiguousarray(wqk).astype(f16),
            "wv16": np.ascontiguousarray(wv_stack).astype(f16),
            "wo01": np.ascontiguousarray(
                Wo[hh[0] * DH:(hh[0] + 2) * DH, :]).astype(f16),
            "wo2": np.ascontiguousarray(
                Wo[hh[2] * DH:(hh[2] + 1) * DH, :]).astype(f16),
            "bq": np.ascontiguousarray(np.stack([bq[h] for h in hh], axis=1)
                                       ).astype(f32),
            "bv": np.ascontiguousarray(
                np.tile(np.concatenate([bv[h] for h in hh]), (128, 1))
            ).astype(f32),
            "ident": np.eye(128, dtype=f16),
        })
    return in_maps


def gather(results, bo):
    out = np.empty((B, S, D), np.float32)
    for b in range(B):
        acc = results[b * CORES_PER_BATCH]["out"].astype(np.float32)
        for c in range(b * CORES_PER_BATCH + 1, (b + 1) * CORES_PER_BATCH):
            acc += results[c]["out"].astype(np.float32)
        out[b] = acc + bo[None, :].astype(np.float32)
    return out


def kernel(x, Wq, Wk, Wv, bq, bk, bv, Wo, bo, c=0, **_unused):
    x, Wq, Wk, Wv, bq, bk, bv, Wo, bo = (
        np.asarray(a, np.float32) for a in (x, Wq, Wk, Wv, bq, bk, bv, Wo, bo)
    )
    nc = _get_module()
    in_maps = make_in_maps(x, Wq, Wk, Wv, bq, bk, bv, Wo)
    res = run_bass_kernel_spmd(nc, in_maps, list(range(NCORES)))
    return gather(res.results, bo)


# revision 66
# speedup vs baseline: 1.0086x; 1.0086x over previous
"""Multi-head attention kernel for Trainium2, sharded over 8 NeuronCores.

Sharding: data parallel over batch (B=2 -> 4 cores each) x tensor parallel
over heads (12 heads -> 3 heads per core). Per-head partial output
projections are summed on the host (the all-reduce of the tensor-parallel
hint) and the output bias added there.

Engine plan (v3 — fp16 everywhere, dual-engine exp, flipped probs@V):
  - all matmul operands are fp16 (1 cyc/col on the PE, same rate as fp32r,
    half the SBUF/DMA bytes). fp8 variants of any tensor feeding the
    softmax-weighted sum were measured 2-3x OVER the 2e-2 gate: quantization
    noise on q/k/v/probs does NOT average away under softmax, because the
    weighted-mean signal and the weighted-mean noise shrink identically.
  - the key bias is dropped entirely: softmax over keys is invariant to the
    per-query constant q.bk, so only the query bias bq survives (it rides
    the forced PSUM->SBUF conversion as a per-partition scalar add).
  - exp splits across BOTH the Activation engine (native Exp -> fp16) and
    the Vector engine (Schraudolph int16 trick: bits = rint(184.665*s_raw
    + 15315) bitcast fp16 ~ exp(s_raw/8), one tensor_scalar op, ~1.7% rms),
    interleaved within each block so both engines run concurrently.
  - probs@V runs FLIPPED: [128,128] probs blocks are the stationary operand
    (LdWeights is free) and the [64 v | ones] fp16 tile streams 65 moving
    columns -> 50k PE cycles instead of 98k for the probs-moving
    orientation. The ones column lands the softmax denominator at psum
    column 64 = per-QUERY-partition, so normalization is one reciprocal +
    one broadcast-multiply per block.
  - the normalized ctx [query, (head,dh)] is transposed back to [dh, query]
    for the output projection via PE permutation matmuls (is_transpose with
    an identity moving operand, 128 cycles per [128,128] block) + engine
    copies. (An SBUF->SBUF dma_start_transpose variant measured faster on
    paper but was flaky on the execution path; PE transposes are stable.)
  - out tiles are copied PSUM->SBUF as fp16 by the Activation engine and
    DMA'd at half the bytes; the host gather sums the 4 tensor-parallel
    partials in fp32 and adds the output bias.
"""

from collections import deque

import numpy as np
import ml_dtypes

import concourse.mybir as mybir
from concourse import bacc
from concourse.tile import TileContext
from concourse.bass_utils import run_bass_kernel_spmd

H, D, DH = 12, 768, 64
B, S = 2, 2048
NCORES = 8
CORES_PER_BATCH = 4
HPC = 3  # heads per core
SQ = 512  # query-chunk width
NSQ = S // SQ  # 4
NSK = S // 128  # 16 key chunks
NPD = 3  # d-chunk pairs (contraction 768 = 3 * 2 * 128)
NPK = NSK // 2  # 8 key-chunk pairs for probs@V DoubleRow

F32 = mybir.dt.float32
F32R = mybir.dt.float32r
F16 = mybir.dt.float16
F8E4 = mybir.dt.float8e4
F8E5 = mybir.dt.float8e5
I16 = mybir.dt.int16
ADD = mybir.AluOpType.add
MULT = mybir.AluOpType.mult
EXP = mybir.ActivationFunctionType.Exp
IDENT = mybir.ActivationFunctionType.Identity
DR = mybir.MatmulPerfMode.DoubleRow

# Schraudolph fp16 exp constants (round-to-nearest float->int16 verified):
# bits = rint(1024*log2(e)/8 * s_raw + bias) => fp16(bits) ~ exp(s_raw/8),
# max rel err 3.0%, ~1.7% rms -> ~0.6% on the final output (fp8 variants
# measured 2-3x over the 2e-2 absmax gate; quantization noise on probs/v
# does NOT average away under softmax - noise and signal shrink identically)
SCH_A = 184.6649652337873
SCH_B = 15315.0

# per-block exp engine pattern (A=Activation native exp, D=DVE int8 trick),
# interleaved so adjacent score tiles land on different engines
EXP_PATS = ["ADADAADA", "ADADADAD", "ADADADAD", "ADADADAD", "ADADADAD", "ADADADAD"]


NDC = D // 128  # 6


def _build_module():
    nc = bacc.Bacc("TRN2", target_bir_lowering=False, debug=False, num_devices=NCORES)
    xT16 = nc.declare_dram_parameter("xT16", [128, NDC, S], F16, isOutput=False)
    wqk16 = nc.declare_dram_parameter("wqk16", [128, HPC, NDC, 128], F16, isOutput=False)
    wv16 = nc.declare_dram_parameter("wv16", [128, NDC, HPC * DH], F16, isOutput=False)
    wo01 = nc.declare_dram_parameter("wo01", [128, D], F16, isOutput=False)
    wo2 = nc.declare_dram_parameter("wo2", [64, D], F16, isOutput=False)
    bq = nc.declare_dram_parameter("bq", [64, HPC], F32, isOutput=False)
    bv = nc.declare_dram_parameter("bv", [128, HPC * DH], F32, isOutput=False)
    ident = nc.declare_dram_parameter("ident", [128, 128], F16, isOutput=False)
    out = nc.declare_dram_parameter("out", [S, D], F16, isOutput=True)

    with TileContext(nc) as tc:
        _body(nc, tc, xT16, wqk16, wv16, wo01, wo2, bq, bv, ident, out)
    nc.compile()
    return nc


def _body(nc, tc, xT16, wqk16, wv16, wo01, wo2, bq, bv, ident, out):
    with (
        tc.tile_pool(name="persist", bufs=1) as P1,
        tc.tile_pool(name="work", bufs=4) as W2,
        tc.tile_pool(name="probs", bufs=2) as PR,
        # PSUM: ACC 2 banks rotating [128,512] tiles (the probs@V
        # accumulator cxp shares this rotation - it is bank-sized and lives
        # only through the in-block tail), SPS 3x[128,1024]: the 3rd score
        # buffer lets the PE pre-issue scores(j+2) so each exp engine runs
        # back-to-back instead of idling exp(j)+sem+scores per pair.
        tc.tile_pool(name="acc", bufs=2, space="PSUM") as ACC,
        tc.tile_pool(name="sps", bufs=3, space="PSUM") as SPS,
    ):
        xT16_sb = P1.tile([128, NDC, S], F16, tag="xT16")
        wqk_sb = P1.tile([128, HPC, NDC, 128], F16, tag="wqk")
        wv_sb = P1.tile([128, NDC, HPC * DH], F16, tag="wv")
        wo01_sb = P1.tile([128, D], F16, tag="wo01")
        wo2_sb = P1.tile([64, D], F16, tag="wo2")
        bq_sb = P1.tile([64, HPC], F32, tag="bq")
        bv_sb = P1.tile([128, HPC * DH], F32, tag="bv")
        id_sb = P1.tile([128, 128], F16, tag="ident")
        # q/k transposed per head, fp16 (scores run fp16 at 1 cyc/col)
        qT = [P1.tile([64, S], F16, tag=f"qT{h}", name=f"qT{h}") for h in range(HPC)]
        kT = [P1.tile([64, S], F16, tag=f"kT{h}", name=f"kT{h}") for h in range(HPC)]
        # v tiles fp16: per key chunk, per head [v_h (64) | ones-column] —
        # the ones column rides the probs@V moving operand and lands the
        # softmax denominator at psum column 64, i.e. per-QUERY-partition
        vp = P1.tile([128, NSK, HPC, 65], F16, tag="vp")

        # DMAs in first-needed order (dma_start costs ~0.6-1us serialized DGE
        # overhead each, so batch big except the first matmul's deps).
        nc.sync.dma_start(xT16_sb[:, :, 0:SQ], xT16[:, :, 0:SQ])
        nc.sync.dma_start(wqk_sb[:, 0, :, :], wqk16[:, 0, :, :])
        nc.sync.dma_start(bq_sb[:], bq[:])
        for sc in range(1, NSQ):
            nc.sync.dma_start(
                xT16_sb[:, :, sc * SQ:(sc + 1) * SQ],
                xT16[:, :, sc * SQ:(sc + 1) * SQ],
            )
        nc.sync.dma_start(wv_sb[:], wv16[:])
        nc.sync.dma_start(wqk_sb[:, 1:3, :, :], wqk16[:, 1:3, :, :])
        nc.sync.dma_start(bv_sb[:], bv[:])
        nc.sync.dma_start(wo01_sb[:], wo01[:])
        nc.sync.dma_start(wo2_sb[:], wo2[:])
        nc.sync.dma_start(id_sb[:], ident[:])
        # ones column next to each head's v block (softmax denominator trick)
        nc.gpsimd.memset(vp[:, :, :, 64:65], 1.0)

        def qk_unit(h, sc):
            # one query-chunk of q/k projection for head h (fp16 matmuls —
            # fp8 here puts ~5% correlated error on q that softmax cannot
            # average away), then PSUM->SBUF fp16 conversions.
            ps = ACC.tile([128, SQ], F32, tag="acc", name=f"qkps{h}_{sc}")
            for o in range(NDC):
                nc.tensor.matmul(
                    ps[:],
                    wqk_sb[:, h, o, :],
                    xT16_sb[:, o, sc * SQ:(sc + 1) * SQ],
                    start=(o == 0),
                    stop=(o == NDC - 1),
                )
            # q rows 0..63 with per-dim bias (DVE); k rows 64..127 plain (ACT)
            nc.vector.tensor_scalar(
                qT[h][:, sc * SQ:(sc + 1) * SQ], ps[0:64, :],
                bq_sb[0:64, h:h + 1], None, ADD,
            )
            nc.scalar.copy(kT[h][:, sc * SQ:(sc + 1) * SQ], ps[64:128, :])

        def v_unit(kc):
            # one key-chunk of v for all 3 heads (fp16 matmuls), then
            # bias-add conversion into the fp16 vp tile (DVE).
            ps = ACC.tile([128, SQ], F32, tag="acc", name=f"vps{kc}")
            psv = ps[:, 0:HPC * DH]
            for o in range(NDC):
                nc.tensor.matmul(
                    psv,
                    xT16_sb[:, o, kc * 128:(kc + 1) * 128],
                    wv_sb[:, o, :],
                    start=(o == 0),
                    stop=(o == NDC - 1),
                )
            nc.vector.tensor_tensor(
                vp[:, kc, :, 0:64],
                psv.rearrange("p (h m) -> p h m", m=DH),
                bv_sb[:].rearrange("p (h m) -> p h m", m=DH),
                ADD,
            )

        def proj_span(sc, ms, n0, nw, ot, ctx01, ctx2, last):
            # one column-span of the out-projection for row-tile ms: both
            # head-accumulation stages into ONE psum tile (peak 1 ACC slot)
            ops_t = ACC.tile([128, nw], F32, tag="acc", name=f"ops{sc}_{ms}_{n0}")
            nc.tensor.matmul(
                ops_t[:], ctx01[:, ms * 128:(ms + 1) * 128],
                wo01_sb[:, n0:n0 + nw], start=True, stop=False,
            )
            nc.tensor.matmul(
                ops_t[:], ctx2[0:64, ms * 128:(ms + 1) * 128],
                wo2_sb[:, n0:n0 + nw], start=False, stop=True,
            )
            nc.scalar.copy(ot[:, n0:n0 + nw], ops_t[:])
            if last:
                nc.sync.dma_start(
                    out[(sc * 4 + ms) * 128:(sc * 4 + ms + 1) * 128, :], ot[:]
                )

        filler = deque()
        # probs@V / normalize / transpose units of block b, woven into block
        # b+1 as PE/vector fillers: a PSUM start_tensor_calc zeroes a whole
        # 2KB bank, so the 4 query-chunk accumulation groups sharing the cxp
        # bank must run one-after-another (qc-outer), which needs the full
        # block's probs — hence the one-block lag.
        pending = deque()
        exp_cnt = [0]

        def attention_block(sc, h, ctxA, ctxB, pops_per_j=1, pop_stride=1,
                            pre_sps=None, nxt=None):
            # probs fp16 flat [128, NSK*SQ], key chunks on partitions; exp
            # runs 1024 wide per key-chunk pair, alternating ACT (native
            # exp) / DVE (int16 Schraudolph). probs@V runs FLIPPED: probs
            # [128,128] blocks are the stationary operand and the [64 v |
            # ones] tile streams 65 columns — half the PE cycles of the
            # probs-moving orientation (and the denominator lands per-QUERY-
            # partition at psum column 64, so normalization is one
            # broadcast-multiply instead of a partition-aligned divide).
            probs = PR.tile([128, NSK * SQ], F16, tag="probs", name=f"pr{sc}_{h}")
            # full-bank cxp so start_tensor_calc's bank-wide zero marking
            # stays within this tile
            cxp = ACC.tile([128, 4, 128], F32, tag="acc", name=f"cxp{sc}_{h}")

            def pvq(qc):
                # one query-window's full key accumulation (qc-outer: groups
                # sharing the bank must not interleave their start/stop)
                for mk in range(NSK):
                    nc.tensor.matmul(
                        cxp[:, qc, 0:65],
                        probs[:, mk * SQ + qc * 128:mk * SQ + (qc + 1) * 128],
                        vp[:, mk, h, :],
                        start=(mk == 0),
                        stop=(mk == NSK - 1),
                    )

            def normalize():
                # cols 0..63: raw ctx [query, dh]; col 64: denominators
                rinv = W2.tile([128, 4, 1], F32, tag="recip", name=f"r{sc}_{h}")
                nc.vector.reciprocal(rinv[:], cxp[:, :, 64:65])
                dst = (ctxA[:, :, h * 64:(h + 1) * 64] if h < 2
                       else ctxB[:, :, 0:64])
                nc.vector.scalar_tensor_tensor(
                    dst, cxp[:, :, 0:64], 1.0,
                    rinv[:].to_broadcast([128, 4, DH]), MULT, MULT,
                )

            npre = len(pre_sps) if pre_sps else 0
            for j in range(NPK):
                if j < npre:
                    sps = pre_sps[j]
                else:
                    sps = SPS.tile([128, 2 * SQ], F32, tag="sps",
                                   name=f"sps{sc}_{h}_{j}")
                    for half in range(2):
                        mk = 2 * j + half
                        nc.tensor.matmul(
                            sps[:, half * SQ:(half + 1) * SQ],
                            kT[h][:, mk * 128:(mk + 1) * 128],
                            qT[h][:, sc * SQ:(sc + 1) * SQ],
                            start=True,
                            stop=True,
                        )
                dst = probs[:, j * 2 * SQ:(j + 1) * 2 * SQ]
                # ACT/DVE interleaved within the block so both engines chew
                # concurrently through the 2-deep score-buffer pipeline;
                # 5:3 / 5:3 / 4:4 block cycle balances total load (~56A/40D)
                pat = EXP_PATS[(exp_cnt[0] // NPK) % len(EXP_PATS)]
                if pat[j] == "A":
                    nc.scalar.activation(dst, sps[:], EXP, scale=0.125)
                else:
                    nc.vector.tensor_scalar(
                        dst.bitcast(I16), sps[:], SCH_A, SCH_B, MULT, ADD
                    )
                exp_cnt[0] += 1
                if j % pop_stride == 0:
                    for _ in range(pops_per_j):
                        if filler:
                            filler.popleft()()
            hoisted = []
            for qc in range(4):
                pvq(qc)
                if qc in (1, 2, 3) and nxt is not None:
                    # hoist the NEXT block's first three score pairs (PE-only
                    # — their exps stay in that block's body): fills this
                    # block's PE tail and lets both exp engines launch at
                    # the next block's start. Touches only the 3-deep sps
                    # rotation and the long-lived qT/kT tiles (the
                    # scores-after-exp WAR the in-block loop already
                    # exercises); hoisting the EXP side corrupts.
                    nsc, nh = nxt
                    p = len(hoisted)
                    hs = SPS.tile([128, 2 * SQ], F32, tag="sps",
                                  name=f"hsps{nsc}_{nh}_{p}")
                    for half in range(2):
                        mk = 2 * p + half
                        nc.tensor.matmul(
                            hs[:, half * SQ:(half + 1) * SQ],
                            kT[nh][:, mk * 128:(mk + 1) * 128],
                            qT[nh][:, nsc * SQ:(nsc + 1) * SQ],
                            start=True,
                            stop=True,
                        )
                    hoisted.append(hs)
            normalize()
            return hoisted

        # PE warmup (p-state ramp) + ACT exp-table preload during DMA wait
        warm = P1.tile([64, 512], F32R, tag="warm")
        nc.vector.memset(warm[:].bitcast(F32), 0.0)
        wps = ACC.tile([128, 512], F32, tag="acc", name="warmps")
        for _ in range(10):
            nc.tensor.matmul(wps[:], warm[:, 0:128], warm[:], start=True, stop=True)
        wact = P1.tile([64, 1], F16, tag="wact")
        nc.scalar.activation(wact[:], warm[:, 0:1].bitcast(F32), EXP, scale=0.125)

        qk_unit(0, 0)

        ctxs = {}
        pre_state = {"sps": None}
        for sc in range(NSQ):
            ctxA = W2.tile([128, 4, 128], F16, tag="ctxA", name=f"cA_{sc}")
            ctxB = W2.tile([128, 4, 128], F16, tag="ctxB", name=f"cB_{sc}")
            c01T = W2.tile([128, SQ], F16, tag="c01T", name=f"c01T_{sc}")
            c2T = W2.tile([128, SQ], F16, tag="c2T", name=f"c2T_{sc}")
            ctxs[sc] = (c01T, c2T)

            def transpose(src, dstT, rows=128):
                # [128 q, qc, hd] -> [hd, qc-window of q] via PE permutation
                # matmuls (is_transpose) + PSUM->SBUF copies
                for qc in range(4):
                    tp = SPS.tile([rows, 128], F16, tag="sps",
                                  name=f"tp{sc}_{qc}_{rows}")
                    nc.tensor.matmul(tp[:], src[:, qc, 0:rows], id_sb[:],
                                     is_transpose=True)
                    if rows == 128:
                        nc.scalar.copy(
                            dstT[0:rows, qc * 128:(qc + 1) * 128], tp[:])
                    else:
                        nc.vector.tensor_copy(
                            dstT[0:rows, qc * 128:(qc + 1) * 128], tp[:])

            for h in range(HPC):
                pops = 1
                if sc == 0 and h == 0:
                    filler.append(lambda: qk_unit(0, 1))
                    filler.append(lambda: qk_unit(0, 2))
                    filler.append(lambda: qk_unit(0, 3))
                    for i in range(NSK // 2):
                        filler.append(lambda i=i: v_unit(2 * i))
                        filler.append(lambda i=i: v_unit(2 * i + 1))
                        if i % 2 == 0:
                            filler.append(lambda i=i: qk_unit(1, i // 2))
                    pops = 3
                elif sc == 0 and h == 1:
                    for i in range(NSQ):
                        filler.append(lambda i=i: qk_unit(2, i))
                stride = 3 if sc > 0 else (3 if h == 1 else 1)
                bidx = sc * HPC + h
                nxt = ((sc + (h + 1) // HPC, (h + 1) % HPC)
                       if bidx + 1 < NSQ * HPC else None)
                pre_state["sps"] = attention_block(
                    sc, h, ctxA, ctxB, pops_per_j=pops, pop_stride=stride,
                    pre_sps=pre_state["sps"], nxt=nxt)
                if h == 1:
                    transpose(ctxA, c01T)
                elif h == 2:
                    transpose(ctxB, c2T, rows=64)
            for ms in range(SQ // 128):
                ot = W2.tile([128, D], F16, tag="out", name=f"ot{sc}_{ms}")
                for n0, nw, last in ((0, 512, False), (512, 256, True)):
                    filler.append(
                        lambda sc=sc, ms=ms, n0=n0, nw=nw, ot=ot, last=last:
                        proj_span(sc, ms, n0, nw, ot, *ctxs[sc], last)
                    )
        while filler:
            filler.popleft()()


_CACHE = {}


def _get_module():
    if "nc" not in _CACHE:
        _CACHE["nc"] = _build_module()
    return _CACHE["nc"]


def make_in_maps(x, Wq, Wk, Wv, bq, bk, bv, Wo):
    f32 = np.float32
    f16 = np.float16
    f8 = ml_dtypes.float8_e4m3
    in_maps = []
    for c in range(NCORES):
        b = c // CORES_PER_BATCH
        hh = [HPC * (c % CORES_PER_BATCH) + i for i in range(HPC)]
        # xT16 [128, chunk, S]: d = chunk*128 + partition
        xt16 = x[b].T.reshape(NDC, 128, S).transpose(1, 0, 2)
        # wqk16 [128, head, chunk, 128]
        wqk = np.stack(
            [np.concatenate([Wq[h], Wk[h]], axis=1) for h in hh]
        )  # [3, 768, 128]
        wqk = wqk.reshape(HPC, NDC, 128, 128).transpose(2, 0, 1, 3)
        # wv16 [128, chunk, 192]
        wv_stack = np.concatenate([Wv[h] for h in hh], axis=1)  # [768, 192]
        wv_stack = wv_stack.reshape(NDC, 128, HPC * DH).transpose(1, 0, 2)
        in_maps.append({
            "xT16": np.ascontiguousarray(xt16).astype(f16),
            "wqk16": np.ascontiguousarray(wqk).astype(f16),
            "wv16": np.ascontiguousarray(wv_stack).astype(f16),
            "wo01": np.ascontiguousarray(
                Wo[hh[0] * DH:(hh[0] + 2) * DH, :]).astype(f16),
            "wo2": np.ascontiguousarray(
                Wo[hh[2] * DH:(hh[2] + 1) * DH, :]).astype(f16),
            "bq": np.ascontiguousarray(np.stack([bq[h] for h in hh], axis=1)
                                       ).astype(f32),
            "bv": np.ascontiguousarray(
                np.tile(np.concatenate([bv[h] for h in hh]), (128, 1))
            ).astype(f32),
            "ident": np.eye(128, dtype=f16),
        })
    return in_maps


def gather(results, bo):
    out = np.empty((B, S, D), np.float32)
    for b in range(B):
        acc = results[b * CORES_PER_BATCH]["out"].astype(np.float32)
        for c in range(b * CORES_PER_BATCH + 1, (b + 1) * CORES_PER_BATCH):
            acc += results[c]["out"].astype(np.float32)
        out[b] = acc + bo[None, :].astype(np.float32)
    return out


def kernel(x, Wq, Wk, Wv, bq, bk, bv, Wo, bo, c=0, **_unused):
    x, Wq, Wk, Wv, bq, bk, bv, Wo, bo = (
        np.asarray(a, np.float32) for a in (x, Wq, Wk, Wv, bq, bk, bv, Wo, bo)
    )
    nc = _get_module()
    in_maps = make_in_maps(x, Wq, Wk, Wv, bq, bk, bv, Wo)
    res = run_bass_kernel_spmd(nc, in_maps, list(range(NCORES)))
    return gather(res.results, bo)
